# revision 28
# baseline (speedup 1.0000x reference)
"""Trainium2 Bass kernel for pointer-generator additive attention.

Full op (per batch b):
    dec_fea = s_t_hat @ W_d.T + b_d                         # (n,)
    att     = EF[b] + dec_fea[None,:] + cov[b][:,None]*W_c  # (t, n)
    score   = tanh(att) @ v                                 # (t,)
    attn    = renorm(softmax(score) * mask)                 # (t,)
    c_t     = attn @ EO[b]                                  # (n,)
    cov_next= cov + attn

Data-parallel over batch across 8 NeuronCores (8 batches/core, params
replicated, no collectives).  Measured ~172 us vs ~220-257 us for the
identity-matmul baseline.  Key design points:

  - T is tiled in chunks of 119 rows so K = 119 (identity band)
    + 8 (one-hot dec-row selectors) + 1 (cov -> W_c) = 128: a SINGLE
    fused matmul per (chunk, n-half) computes EF + dec_fea + cov*W_c.
    The lhsT matrix (identity + one-hots + cov row) is built on the
    host; dec rows live at partitions 119..126 of the EF buffers
    (restored by one small DMA per batch since the EF block DMA
    overwrites them), W_c at partition 127 (staged by the host).
  - EF and the att-matmul lhsT are staged in FP8-E4M3 (EO in bf16):
    HBM traffic drops to ~30 MB/core and measured rel_err is 1.3e-2
    against the 2e-2 tolerance.  EO stays bf16 (fp8 EO would put c_t
    at ~4.5e-2).
  - DMA pattern rule learned by microbenchmark: transfers must write
    ALL 128 partitions with p-minor (row-interleaved) DRAM order --
    that runs at ~340-370 B/ns; 119-partition or per-partition-
    contiguous patterns run at ~100-160 B/ns.  EF/EO are therefore
    staged as [9, 128, N] zero-padded blocks.
  - ScalarE: one tanh per chunk (PSUM f32 -> SBUF bf16); VectorE:
    scalar_tensor_tensor with accum_out gives the n-reduction per
    chunk.  Scores regroup to [1, T] rows via one PE transpose and
    two small DMAs per batch.
  - Masked softmax + renorm + coverage batched in GROUPS OF 4 on
    [4, 1024] row tiles (engine partition bases must be 32-aligned,
    so per-group tiles, not [8, T] slices).
  - c_t: per chunk, lhsT = [119, 8] one-hot-column attn (column b =
    attn chunk, rest zeros) so all 8 batches accumulate into a single
    [8, 512] x2 PSUM group; one ScalarE copy + one DMA at the end.
  - Schedule: pass-1 group 0 -> softmax(0) -> {pass-1 group 1
    interleaved with pass-2 group 0} -> softmax(1) -> pass-2 group 1,
    with EF prefetched 2 batches and EO 2-3 buffers ahead, so the PE
    and the DMA engines stay busy across the softmax barriers.
"""

import sys

if "/opt/trn_rl_repo" not in sys.path:
    sys.path.insert(0, "/opt/trn_rl_repo")

import ml_dtypes
import numpy as np

import concourse.bass as bass
import concourse.mybir as mybir
import concourse.tile as tile
from concourse import bacc
from concourse.bass_utils import run_bass_kernel_spmd
from concourse.masks import make_identity

F32 = mybir.dt.float32
BF16 = mybir.dt.bfloat16
FP8 = mybir.dt.float8e4
AF = mybir.ActivationFunctionType
ALU = mybir.AluOpType

N_CORES = 8
B = 64
NB = B // N_CORES  # local batches per core
T = 1024
N = 1024
CH = 119           # t-chunk height (identity rows in the fused matmul)
NCH = 9            # chunks per batch: 8*119 + 72
LAST = T - (NCH - 1) * CH  # 72
W = NCH * CH       # 1071: per-batch window stride in lhsT
KT = N // 128      # k-tiles for the W_d matvec
GRP = 4            # softmax group size


def build_bass(nb: int = NB) -> bass.Bass:
    nc = bacc.Bacc()

    ef_d = nc.declare_dram_parameter("ef_blk", [nb, NCH, 128, N], FP8, isOutput=False)
    eo_d = nc.declare_dram_parameter("eo_blk", [nb, 8, 128, N], BF16, isOutput=False)
    lhsA_d = nc.declare_dram_parameter("lhsT_a", [4, 128, 2048], FP8, isOutput=False)
    lhsB_d = nc.declare_dram_parameter("lhsT_b", [128, nb * W - 8192], FP8, isOutput=False)
    mk_d = nc.declare_dram_parameter("enc_padding_mask", [nb, T], F32, isOutput=False)
    cv_d = nc.declare_dram_parameter("coverage", [nb, T], F32, isOutput=False)
    wdt_d = nc.declare_dram_parameter("W_d_T", [4, 128, 2048], BF16, isOutput=False)
    st_d = nc.declare_dram_parameter("s_t_hat_T", [N, nb], BF16, isOutput=False)
    bd_d = nc.declare_dram_parameter("b_d", [N], BF16, isOutput=False)
    wc_d = nc.declare_dram_parameter("W_c", [N], BF16, isOutput=False)
    v_d = nc.declare_dram_parameter("v", [N], BF16, isOutput=False)
    ct_o = nc.declare_dram_parameter("c_t", [nb, N], F32, isOutput=True)
    at_o = nc.declare_dram_parameter("attn", [nb, T], F32, isOutput=True)
    cn_o = nc.declare_dram_parameter("coverage_next", [nb, T], F32, isOutput=True)

    with tile.TileContext(nc) as tc:
        with (
            tc.tile_pool(name="consts", bufs=1) as consts,
            tc.tile_pool(name="lhsp", bufs=1) as lhsp,
            tc.tile_pool(name="wdtp", bufs=4) as wdtp,
            tc.tile_pool(name="efp", bufs=3) as efp,
            tc.tile_pool(name="eop", bufs=3) as eop,
            tc.tile_pool(name="thp", bufs=3) as thp,
            tc.tile_pool(name="ttro", bufs=2) as ttro,
            tc.tile_pool(name="smal", bufs=2) as smal,
            tc.tile_pool(name="a9p", bufs=3) as a9p,
            tc.tile_pool(name="acwp", bufs=3) as acwp,
            tc.tile_pool(name="psA", bufs=2, space="PSUM") as psA,
            tc.tile_pool(name="psS", bufs=2, space="PSUM") as psS,
            tc.tile_pool(name="psT", bufs=2, space="PSUM") as psT,
        ):
            # ---------------- constants / small inputs ----------------
            ident = consts.tile([CH, CH], F32)
            make_identity(nc, ident)

            # dec matvec inputs stream first on the sync queue so the
            # dec -> restore(0) chain clears as early as possible
            sT_all = consts.tile([128, KT, NB], BF16)
            nc.sync.dma_start(
                out=sT_all, in_=st_d.rearrange("(kj p) b -> p kj b", p=128)
            )
            wpairs = []
            for c2 in range(4):
                wpair = wdtp.tile([128, 2, N], BF16, tag="wp", name=f"wp{c2}")
                nc.sync.dma_start(
                    out=wpair, in_=wdt_d[c2, :, :].rearrange("p (k n) -> p k n", k=2)
                )
                wpairs.append(wpair)

            bd_b = consts.tile([1, N], BF16)
            nc.sync.dma_start(out=bd_b, in_=bd_d[None, :])
            v_b = consts.tile([1, N], BF16)
            nc.sync.dma_start(out=v_b, in_=v_d[None, :])
            wc_b = consts.tile([1, N], BF16)
            nc.sync.dma_start(out=wc_b, in_=wc_d[None, :])
            ones8 = consts.tile([1, NB], BF16)
            nc.vector.memset(ones8, 1.0)

            # first-batch EF blocks load before anything else on gpsimd,
            # then batch 1 in full, then the lhsT constant
            ef_bufs_early = [
                efp.tile([128, NCH, N], FP8, tag="ef", name=f"efb{i}e")
                for i in range(2)]
            for c in range(NCH):
                nc.gpsimd.dma_start(out=ef_bufs_early[0][:, c, :],
                                    in_=ef_d[0, c, :, :])
            nc.gpsimd.dma_start(
                out=ef_bufs_early[1][:, :, :],
                in_=ef_d[1, :, :, :].rearrange("c p n -> p c n"),
            )

            # full lhsT (identity band + one-hot dec selectors + cov row)
            lhs_all = lhsp.tile([128, nb * W], FP8)
            nc.gpsimd.dma_start(
                out=lhs_all[:, 0:8192].rearrange("p (c m) -> p c m", c=4),
                in_=lhsA_d[:, :, :].rearrange("c p m -> p c m"),
            )
            nc.gpsimd.dma_start(out=lhs_all[:, 8192:], in_=lhsB_d[:, :])

            # softmax row tiles: set A serves batches 0-3 and is then
            # reused for batches 6-7 (their masks/coverage load separately);
            # set B serves batches 4-5.  Engine partition bases must be
            # 32-aligned, so every set starts at partition 0.
            mask_A = consts.tile([GRP, T], F32)
            nc.sync.dma_start(out=mask_A, in_=mk_d[0:4, :])
            cov_A = consts.tile([GRP, T], F32)
            nc.sync.dma_start(out=cov_A, in_=cv_d[0:4, :])
            score_A = consts.tile([GRP, T], F32)
            attn_A = consts.tile([GRP, T], F32)
            covn_A = consts.tile([GRP, T], F32)
            mask_B = consts.tile([2, T], F32)
            nc.sync.dma_start(out=mask_B, in_=mk_d[4:6, :])
            cov_B = consts.tile([2, T], F32)
            nc.sync.dma_start(out=cov_B, in_=cv_d[4:6, :])
            score_B = consts.tile([2, T], F32)
            attn_B = consts.tile([2, T], F32)
            covn_B = consts.tile([2, T], F32)
            mask_C = consts.tile([2, T], F32)
            nc.sync.dma_start(out=mask_C, in_=mk_d[6:8, :])
            cov_C = consts.tile([2, T], F32)
            nc.sync.dma_start(out=cov_C, in_=cv_d[6:8, :])

            def sm_row(b):
                # (tile-row holding batch b's score/attn, row index)
                if b < 4:
                    return score_A, attn_A, b
                if b < 6:
                    return score_B, attn_B, b - 4
                return score_A, attn_A, b - 6

            ct_sb = consts.tile([nb, N], F32)

            # v broadcast to all partitions for the score reduction
            v_bcast = consts.tile([128, N], BF16)
            nc.gpsimd.partition_broadcast(v_bcast, v_b)

            # dec_fea rows = s_t_hat @ W_d.T + b_d  (k-pairs as staged)
            dec_rows = consts.tile([NB, N], BF16)
            psd = [psA.tile([NB, 512], F32, tag="att", name=f"psd{h}")
                   for h in range(2)]
            for c2 in range(4):
                for k in range(2):
                    kj = 2 * c2 + k
                    for h in range(2):
                        nc.tensor.matmul(
                            psd[h],
                            lhsT=sT_all[:, kj, :],
                            rhs=wpairs[c2][:, k, h * 512:(h + 1) * 512],
                            start=(kj == 0), stop=False,
                            skip_group_check=True,
                        )
            for h in range(2):
                sl = slice(h * 512, (h + 1) * 512)
                nc.tensor.matmul(
                    psd[h], lhsT=ones8, rhs=bd_b[0:1, sl],
                    start=False, stop=True, skip_group_check=True,
                )
                nc.scalar.activation(dec_rows[:, sl], psd[h], AF.Copy)

            # EF stream buffers: partitions 119..126 = dec rows,
            # partition 127 = W_c (constant across batches/chunks)
            ef_bufs = ef_bufs_early + [
                efp.tile([128, NCH, N], FP8, tag="ef", name="efb2")]
            eo_bufs_pool = [eop.tile([128, 8, N], BF16, tag="eo", name=f"eob{i}")
                            for i in range(3)]
            # dec rows replicated across chunks for the per-batch restore DMA
            dec_wide = consts.tile([NB, NCH, N], FP8)
            for c in range(NCH):
                nc.vector.tensor_copy(dec_wide[:, c, :], dec_rows)

            # ---------------- pass 1: scores ----------------
            def chunk_m(c):
                return LAST if c == NCH - 1 else CH

            score_cols_t = {}

            def phase_a(b):
                buf = ef_bufs[b % 3]
                if b > 1:
                    nc.gpsimd.dma_start(
                        out=buf[:, :, :],
                        in_=ef_d[b, :, :, :].rearrange("c p n -> p c n"),
                    )
                # the block DMA zeroes partitions 119..126; restore dec rows
                # (W_c at partition 127 is staged by the host)
                if b == 0:
                    for c in range(NCH):
                        nc.sync.dma_start(
                            out=buf[119:127, c, :], in_=dec_wide[:, c, :]
                        )
                else:
                    nc.sync.dma_start(out=buf[119:127, :, :], in_=dec_wide)

                score_cols = smal.tile([CH, NCH], F32, tag="scol")
                score_cols_t[b] = score_cols
                for c in range(NCH):
                    m = chunk_m(c)
                    att = psA.tile([CH, N], F32, tag="att")
                    for h in range(2):
                        nc.tensor.matmul(
                            att[0:m, h * 512:(h + 1) * 512],
                            lhsT=lhs_all[:, b * W + c * CH: b * W + c * CH + m],
                            rhs=buf[:, c, h * 512:(h + 1) * 512],
                            start=True, stop=True, skip_group_check=True,
                        )
                    th = thp.tile([CH, N], BF16, tag="th")
                    nc.scalar.activation(th[0:m, :], att[0:m, :], AF.Tanh)
                    scr = ttro.tile([CH, N], BF16, tag="ttro")
                    nc.vector.scalar_tensor_tensor(
                        out=scr[0:m, :], in0=th[0:m, :], scalar=1.0,
                        in1=v_bcast[0:m, :],
                        op0=ALU.mult, op1=ALU.mult,
                        accum_out=score_cols[0:m, c:c + 1],
                    )

                # score columns -> row b of score_all (t = c*119 + p)
                ps9 = psT.tile([NCH, CH], F32, tag="tscratch")
                nc.tensor.matmul(
                    ps9, lhsT=score_cols, rhs=ident, is_transpose=True,
                    start=True, stop=True,
                )
                score9 = smal.tile([NCH, CH], F32, tag="s9")
                nc.scalar.activation(score9, ps9, AF.Copy)
                srow, _, r = sm_row(b)
                nc.sync.dma_start(
                    out=srow[r:r + 1, 0:(NCH - 1) * CH].rearrange(
                        "p (c w) -> p c w", c=NCH - 1),
                    in_=score9[0:NCH - 1, :],
                )
                nc.sync.dma_start(
                    out=srow[r:r + 1, (NCH - 1) * CH:T],
                    in_=score9[NCH - 1:NCH, 0:LAST],
                )

            # masked softmax + renorm + coverage for GRP batches at once
            def softmax(score_t, attn_t, mask_t, cov_t, covn_t, nr):
                # scores are O(1) (|s| < ~3): plain exp is safe, skip max-sub
                sl = slice(0, nr)
                nc.scalar.activation(attn_t[sl, :], score_t[sl, :], AF.Exp)
                ssum = smal.tile([GRP, 1], F32, tag="ssum")
                nc.vector.scalar_tensor_tensor(
                    out=attn_t[sl, :], in0=attn_t[sl, :], scalar=1.0,
                    in1=mask_t[sl, :],
                    op0=ALU.mult, op1=ALU.mult, accum_out=ssum[sl, :],
                )
                rs = smal.tile([GRP, 1], F32, tag="rs")
                nc.vector.reciprocal(rs[sl, :], ssum[sl, :])
                nc.vector.tensor_scalar_mul(attn_t[sl, :], attn_t[sl, :],
                                            rs[sl, :])
                nc.vector.tensor_add(covn_t[sl, :], cov_t[sl, :],
                                     attn_t[sl, :])

            # ---------------- pass 2: context vectors ----------------
            eo_bufs = {}

            def load_eo(b):
                buf = eo_bufs_pool[b % 3]
                nc.sync.dma_start(
                    out=buf[:, :, :],
                    in_=eo_d[b, :, :, :].rearrange("c p n -> p c n"),
                )
                eo_bufs[b] = buf

            ctps = [psS.tile([NB, 512], F32, tag="srow", name=f"ctp{h}")
                    for h in range(2)]

            acw_t = {}

            def prep_c(b):
                # attn row -> [8, 128] -> transpose -> one-hot column b
                # (emitted ahead of later batches' score scatters to avoid
                # sync-queue head-of-line blocking)
                attn8 = a9p.tile([8, 128], F32, tag="attn9")
                _, arow, r = sm_row(b)
                nc.sync.dma_start(
                    out=attn8,
                    in_=arow[r:r + 1, :].rearrange("p (j t) -> p j t", j=8),
                )
                acp = psT.tile([128, 8], F32, tag="tscratch")
                nc.tensor.matmul(
                    acp, lhsT=attn8, rhs=ident[0:8, 0:8],
                    is_transpose=True, start=True, stop=True,
                )
                acw = acwp.tile([128, 8, NB], BF16, tag="acw")
                nc.gpsimd.memset(acw, 0.0)
                nc.scalar.activation(acw[:, :, b], acp, AF.Copy)
                acw_t[b] = acw

            def mm_c(b):
                acw = acw_t.pop(b)
                buf = eo_bufs.pop(b)
                for c in range(8):
                    for h in range(2):
                        nc.tensor.matmul(
                            ctps[h],
                            lhsT=acw[:, c, :],
                            rhs=buf[:, c, h * 512:(h + 1) * 512],
                            start=(b == 0 and c == 0),
                            stop=(b == nb - 1 and c == 7),
                            skip_group_check=True,
                        )

            # ---------------- schedule ----------------
            # softmax split 0-3 / 4-5 / 6-7: C(4),C(5) depend only on the
            # early 4-5 softmax, so the post-A(7) barrier is just batches
            # 6-7's row ops, fully hidden under C(4)/C(5)
            for b in range(4):
                phase_a(b)
                if b >= 2:
                    load_eo(b - 2)
            softmax(score_A, attn_A, mask_A, cov_A, covn_A, 4)
            # rows 0-3 flush now so set A can be reused for batches 6-7
            nc.sync.dma_start(out=at_o[0:4, :], in_=attn_A)
            nc.sync.dma_start(out=cn_o[0:4, :], in_=covn_A)
            prep_c(0)
            phase_a(4)
            mm_c(0)
            load_eo(2)
            prep_c(1)
            phase_a(5)
            mm_c(1)
            load_eo(3)
            softmax(score_B, attn_B, mask_B, cov_B, covn_B, 2)
            prep_c(2)
            phase_a(6)
            mm_c(2)
            load_eo(4)
            prep_c(3)
            prep_c(4)
            prep_c(5)
            phase_a(7)
            mm_c(3)
            load_eo(5)
            load_eo(6)
            softmax(score_A, attn_A, mask_C, cov_C, covn_A, 2)
            mm_c(4)
            load_eo(7)
            mm_c(5)
            prep_c(6)
            prep_c(7)
            mm_c(6)
            mm_c(7)
            nc.sync.dma_start(out=at_o[4:6, :], in_=attn_B)
            nc.sync.dma_start(out=cn_o[4:6, :], in_=covn_B)
            nc.sync.dma_start(out=at_o[6:8, :], in_=attn_A[0:2, :])
            nc.sync.dma_start(out=cn_o[6:8, :], in_=covn_A[0:2, :])

            for h in range(2):
                nc.scalar.activation(
                    ct_sb[:, h * 512:(h + 1) * 512], ctps[h], AF.Copy
                )
            nc.sync.dma_start(out=ct_o[:, :], in_=ct_sb)

    nc.finalize()
    return nc


_CACHE: dict = {}


def _get_nc() -> bass.Bass:
    if "nc" not in _CACHE:
        _CACHE["nc"] = build_bass(NB)
    return _CACHE["nc"]


def _stage_tiles(x, row127=None, dtype=ml_dtypes.bfloat16):
    """[T, N] f32 -> [9, 128, N] blocks: block c rows 0..118 hold
    t = c*119 + p (zero-padded past T); rows 119..126 zero; row 127 =
    `row127` if given (W_c for the EF operand).  DMA-friendly: every
    transfer writes all 128 partitions with p-minor lines."""
    blk = np.zeros((NCH, 128, N), np.float32)
    pad = np.zeros((CH * NCH, N), np.float32)
    pad[:T] = x
    blk[:, 0:CH, :] = pad.reshape(NCH, CH, N)
    if row127 is not None:
        blk[:, 127, :] = row127[None, :]
    return blk.astype(dtype)


def _build_lhs(cov_core):
    """identity band + one-hot rows + cov row, split into 4KB-line part A
    ([4, 128, 2048], cols 0..8191 pair-packed) and tail part B."""
    lhs = np.zeros((128, NB * W), np.float32)
    eye = np.eye(CH, dtype=np.float32)
    lhs[:CH] = np.tile(eye, (1, NB * NCH))
    for j in range(NB):
        lhs[CH + j, j * W:(j + 1) * W] = 1.0
        lhs[127, j * W:j * W + T] = cov_core[j]
    lhs = lhs.astype(ml_dtypes.float8_e4m3)
    a = np.ascontiguousarray(lhs[:, 0:8192].reshape(128, 4, 2048).transpose(1, 0, 2))
    b = np.ascontiguousarray(lhs[:, 8192:])
    return a, b


def make_in_maps(inputs: dict) -> list:
    f = lambda x: np.ascontiguousarray(np.asarray(x), dtype=np.float32)
    s = f(inputs["s_t_hat"])
    eo = f(inputs["encoder_outputs"])
    ef = f(inputs["encoder_feature"]).reshape(B, T, N)
    mk = f(inputs["enc_padding_mask"])
    cv = f(inputs["coverage"])
    wdt = np.ascontiguousarray(f(inputs["W_d"]).T).astype(ml_dtypes.bfloat16)
    # [1024, 1024] -> k-tile pair layout [4, 128, 2048] (4KB DMA lines):
    # wdt_all[p, kj, :] = W_d_T[kj*128 + p, :]
    wdt_pairs = np.ascontiguousarray(
        wdt.reshape(KT, 128, N).transpose(1, 0, 2).reshape(128, 4, 2048)
        .transpose(1, 0, 2)
    )
    bd = f(inputs["b_d"])
    wc = f(inputs["W_c"])
    vv = f(inputs["v"])
    in_maps = []
    for i in range(N_CORES):
        sl = slice(i * NB, (i + 1) * NB)
        ef_blk = np.stack([_stage_tiles(ef[i * NB + j], row127=wc,
                                        dtype=ml_dtypes.float8_e4m3)
                           for j in range(NB)])
        eo_blk = np.ascontiguousarray(
            eo[sl].reshape(NB, 8, 128, N)).astype(ml_dtypes.bfloat16)
        lhs_a, lhs_b = _build_lhs(cv[sl])
        in_maps.append({
            "ef_blk": ef_blk,
            "eo_blk": eo_blk,
            "lhsT_a": lhs_a,
            "lhsT_b": lhs_b,
            "s_t_hat_T": np.ascontiguousarray(s[sl].T).astype(ml_dtypes.bfloat16),
            "enc_padding_mask": mk[sl],
            "coverage": cv[sl],
            "W_d_T": wdt_pairs,
            "b_d": bd.astype(ml_dtypes.bfloat16),
            "W_c": wc.astype(ml_dtypes.bfloat16),
            "v": vv.astype(ml_dtypes.bfloat16),
        })
    return in_maps


def gather_outputs(results: list):
    c_t = np.concatenate([results[i]["c_t"] for i in range(N_CORES)], axis=0)
    attn = np.concatenate([results[i]["attn"] for i in range(N_CORES)], axis=0)
    covn = np.concatenate(
        [results[i]["coverage_next"] for i in range(N_CORES)], axis=0
    )
    return c_t, attn, covn


def kernel(**inputs):
    nc = _get_nc()
    in_maps = make_in_maps(inputs)
    res = run_bass_kernel_spmd(nc, in_maps, core_ids=list(range(N_CORES)))
    return gather_outputs(res.results)


# revision 30
# speedup vs baseline: 1.1467x; 1.1467x over previous
"""Trainium2 Bass kernel for pointer-generator additive attention.

Full op (per batch b):
    dec_fea = s_t_hat @ W_d.T + b_d                         # (n,)
    att     = EF[b] + dec_fea[None,:] + cov[b][:,None]*W_c  # (t, n)
    score   = tanh(att) @ v                                 # (t,)
    attn    = renorm(softmax(score) * mask)                 # (t,)
    c_t     = attn @ EO[b]                                  # (n,)
    cov_next= cov + attn

Data-parallel over batch across 8 NeuronCores (8 batches/core, params
replicated, no collectives).  Measured ~172 us vs ~220-257 us for the
identity-matmul baseline.  Key design points:

  - T is tiled in chunks of 119 rows so K = 119 (identity band)
    + 8 (one-hot dec-row selectors) + 1 (cov -> W_c) = 128: a SINGLE
    fused matmul per (chunk, n-half) computes EF + dec_fea + cov*W_c.
    The lhsT matrix (identity + one-hots + cov row) is built on the
    host; dec rows live at partitions 119..126 of the EF buffers
    (restored by one small DMA per batch since the EF block DMA
    overwrites them), W_c at partition 127 (staged by the host).
  - EF and the att-matmul lhsT are staged in FP8-E4M3 (EO in bf16):
    HBM traffic drops to ~30 MB/core and measured rel_err is 1.3e-2
    against the 2e-2 tolerance.  EO stays bf16 (fp8 EO would put c_t
    at ~4.5e-2).
  - DMA pattern rule learned by microbenchmark: transfers must write
    ALL 128 partitions with p-minor (row-interleaved) DRAM order --
    that runs at ~340-370 B/ns; 119-partition or per-partition-
    contiguous patterns run at ~100-160 B/ns.  EF/EO are therefore
    staged as [9, 128, N] zero-padded blocks.
  - ScalarE: one tanh per chunk (PSUM f32 -> SBUF bf16); VectorE:
    scalar_tensor_tensor with accum_out gives the n-reduction per
    chunk.  Scores regroup to [1, T] rows via one PE transpose and
    two small DMAs per batch.
  - Masked softmax + renorm + coverage batched in GROUPS OF 4 on
    [4, 1024] row tiles (engine partition bases must be 32-aligned,
    so per-group tiles, not [8, T] slices).
  - c_t: per chunk, lhsT = [119, 8] one-hot-column attn (column b =
    attn chunk, rest zeros) so all 8 batches accumulate into a single
    [8, 512] x2 PSUM group; one ScalarE copy + one DMA at the end.
  - Schedule: pass-1 group 0 -> softmax(0) -> {pass-1 group 1
    interleaved with pass-2 group 0} -> softmax(1) -> pass-2 group 1,
    with EF prefetched 2 batches and EO 2-3 buffers ahead, so the PE
    and the DMA engines stay busy across the softmax barriers.
"""

import sys

if "/opt/trn_rl_repo" not in sys.path:
    sys.path.insert(0, "/opt/trn_rl_repo")

import ml_dtypes
import numpy as np

import concourse.bass as bass
import concourse.mybir as mybir
import concourse.tile as tile
from concourse import bacc
from concourse.bass_utils import run_bass_kernel_spmd
from concourse.masks import make_identity

F32 = mybir.dt.float32
BF16 = mybir.dt.bfloat16
FP8 = mybir.dt.float8e4
AF = mybir.ActivationFunctionType
ALU = mybir.AluOpType

N_CORES = 8
B = 64
NB = B // N_CORES  # local batches per core
T = 1024
N = 1024
CH = 119           # t-chunk height (identity rows in the fused matmul)
NCH = 9            # chunks per batch: 8*119 + 72
LAST = T - (NCH - 1) * CH  # 72
W = NCH * CH       # 1071: per-batch window stride in lhsT
KT = N // 128      # k-tiles for the W_d matvec
GRP = 4            # softmax group size


def build_bass(nb: int = NB) -> bass.Bass:
    nc = bacc.Bacc()

    ef_d = nc.declare_dram_parameter("ef_blk", [nb, NCH, 128, N], FP8, isOutput=False)
    eo_d = nc.declare_dram_parameter("eo_blk", [nb, 8, 128, N], BF16, isOutput=False)
    lhsA_d = nc.declare_dram_parameter("lhsT_a", [4, 128, 2048], FP8, isOutput=False)
    lhsB_d = nc.declare_dram_parameter("lhsT_b", [128, nb * W - 8192], FP8, isOutput=False)
    mk_d = nc.declare_dram_parameter("enc_padding_mask", [nb, T], F32, isOutput=False)
    cv_d = nc.declare_dram_parameter("coverage", [nb, T], F32, isOutput=False)
    wdt_d = nc.declare_dram_parameter("W_d_T", [4, 128, 2048], BF16, isOutput=False)
    st_d = nc.declare_dram_parameter("s_t_hat_T", [N, nb], BF16, isOutput=False)
    bd_d = nc.declare_dram_parameter("b_d", [N], BF16, isOutput=False)
    wc_d = nc.declare_dram_parameter("W_c", [N], BF16, isOutput=False)
    v_d = nc.declare_dram_parameter("v", [N], BF16, isOutput=False)
    ct_o = nc.declare_dram_parameter("c_t", [nb, N], F32, isOutput=True)
    at_o = nc.declare_dram_parameter("attn", [nb, T], F32, isOutput=True)
    cn_o = nc.declare_dram_parameter("coverage_next", [nb, T], F32, isOutput=True)

    with tile.TileContext(nc) as tc:
        with (
            tc.tile_pool(name="consts", bufs=1) as consts,
            tc.tile_pool(name="lhsp", bufs=1) as lhsp,
            tc.tile_pool(name="wdtp", bufs=4) as wdtp,
            tc.tile_pool(name="efp", bufs=3) as efp,
            tc.tile_pool(name="eop", bufs=3) as eop,
            tc.tile_pool(name="thp", bufs=3) as thp,
            tc.tile_pool(name="ttro", bufs=2) as ttro,
            tc.tile_pool(name="smal", bufs=2) as smal,
            tc.tile_pool(name="a9p", bufs=2) as a9p,
            tc.tile_pool(name="acwp", bufs=2) as acwp,
            tc.tile_pool(name="psA", bufs=2, space="PSUM") as psA,
            tc.tile_pool(name="psS", bufs=2, space="PSUM") as psS,
            tc.tile_pool(name="psT", bufs=2, space="PSUM") as psT,
        ):
            # ---------------- constants / small inputs ----------------
            ident = consts.tile([CH, CH], F32)
            make_identity(nc, ident)

            # dec matvec inputs stream first on the sync queue so the
            # dec -> restore(0) chain clears as early as possible
            sT_all = consts.tile([128, KT, NB], BF16)
            nc.sync.dma_start(
                out=sT_all, in_=st_d.rearrange("(kj p) b -> p kj b", p=128)
            )
            wpairs = []
            for c2 in range(4):
                wpair = wdtp.tile([128, 2, N], BF16, tag="wp", name=f"wp{c2}")
                nc.sync.dma_start(
                    out=wpair, in_=wdt_d[c2, :, :].rearrange("p (k n) -> p k n", k=2)
                )
                wpairs.append(wpair)

            bd_b = consts.tile([1, N], BF16)
            nc.sync.dma_start(out=bd_b, in_=bd_d[None, :])
            v_b = consts.tile([1, N], BF16)
            nc.sync.dma_start(out=v_b, in_=v_d[None, :])
            wc_b = consts.tile([1, N], BF16)
            nc.sync.dma_start(out=wc_b, in_=wc_d[None, :])
            ones8 = consts.tile([1, NB], BF16)
            nc.vector.memset(ones8, 1.0)

            # first-batch EF blocks load before anything else on gpsimd,
            # then batch 1 in full, then the lhsT constant
            ef_bufs_early = [
                efp.tile([128, NCH, N], FP8, tag="ef", name=f"efb{i}e")
                for i in range(2)]
            for c in range(NCH):
                nc.gpsimd.dma_start(out=ef_bufs_early[0][:, c, :],
                                    in_=ef_d[0, c, :, :])
            nc.gpsimd.dma_start(
                out=ef_bufs_early[1][:, :, :],
                in_=ef_d[1, :, :, :].rearrange("c p n -> p c n"),
            )

            # full lhsT (identity band + one-hot dec selectors + cov row)
            lhs_all = lhsp.tile([128, nb * W], FP8)
            nc.gpsimd.dma_start(
                out=lhs_all[:, 0:8192].rearrange("p (c m) -> p c m", c=4),
                in_=lhsA_d[:, :, :].rearrange("c p m -> p c m"),
            )
            nc.gpsimd.dma_start(out=lhs_all[:, 8192:], in_=lhsB_d[:, :])

            # softmax row tiles: set A serves batches 0-3 and is then
            # reused for batches 6-7 (their masks/coverage load separately);
            # set B serves batches 4-5.  Engine partition bases must be
            # 32-aligned, so every set starts at partition 0.
            mask_A = consts.tile([GRP, T], F32)
            nc.sync.dma_start(out=mask_A, in_=mk_d[0:4, :])
            cov_A = consts.tile([GRP, T], F32)
            nc.sync.dma_start(out=cov_A, in_=cv_d[0:4, :])
            score_A = consts.tile([GRP, T], F32)
            attn_A = consts.tile([GRP, T], F32)
            covn_A = consts.tile([GRP, T], F32)
            mask_B = consts.tile([2, T], F32)
            nc.sync.dma_start(out=mask_B, in_=mk_d[4:6, :])
            cov_B = consts.tile([2, T], F32)
            nc.sync.dma_start(out=cov_B, in_=cv_d[4:6, :])
            score_B = consts.tile([2, T], F32)
            attn_B = consts.tile([2, T], F32)
            covn_B = consts.tile([2, T], F32)
            mask_C = consts.tile([2, T], F32)
            nc.sync.dma_start(out=mask_C, in_=mk_d[6:8, :])
            cov_C = consts.tile([2, T], F32)
            nc.sync.dma_start(out=cov_C, in_=cv_d[6:8, :])

            def sm_row(b):
                # (tile-row holding batch b's score/attn, row index)
                if b < 4:
                    return score_A, attn_A, b
                if b < 6:
                    return score_B, attn_B, b - 4
                return score_A, attn_A, b - 6

            ct_sb = consts.tile([nb, N], F32)

            # v broadcast to all partitions for the score reduction
            v_bcast = consts.tile([128, N], BF16)
            nc.gpsimd.partition_broadcast(v_bcast, v_b)

            # dec_fea rows = s_t_hat @ W_d.T + b_d  (k-pairs as staged)
            dec_rows = consts.tile([NB, N], BF16)
            psd = [psA.tile([NB, 512], F32, tag="att", name=f"psd{h}")
                   for h in range(2)]
            for c2 in range(4):
                for k in range(2):
                    kj = 2 * c2 + k
                    for h in range(2):
                        nc.tensor.matmul(
                            psd[h],
                            lhsT=sT_all[:, kj, :],
                            rhs=wpairs[c2][:, k, h * 512:(h + 1) * 512],
                            start=(kj == 0), stop=False,
                            skip_group_check=True,
                        )
            for h in range(2):
                sl = slice(h * 512, (h + 1) * 512)
                nc.tensor.matmul(
                    psd[h], lhsT=ones8, rhs=bd_b[0:1, sl],
                    start=False, stop=True, skip_group_check=True,
                )
                nc.scalar.activation(dec_rows[:, sl], psd[h], AF.Copy)

            # EF stream buffers: partitions 119..126 = dec rows,
            # partition 127 = W_c (constant across batches/chunks)
            ef_bufs = ef_bufs_early + [
                efp.tile([128, NCH, N], FP8, tag="ef", name="efb2")]
            eo_bufs_pool = [eop.tile([128, 8, N], BF16, tag="eo", name=f"eob{i}")
                            for i in range(3)]
            # dec rows replicated across chunks for the per-batch restore DMA
            dec_wide = consts.tile([NB, NCH, N], FP8)
            for c in range(NCH):
                nc.vector.tensor_copy(dec_wide[:, c, :], dec_rows)

            # ---------------- pass 1: scores ----------------
            def chunk_m(c):
                return LAST if c == NCH - 1 else CH

            score_cols_t = {}

            def phase_a(b):
                buf = ef_bufs[b % 3]
                if b > 1:
                    nc.gpsimd.dma_start(
                        out=buf[:, :, :],
                        in_=ef_d[b, :, :, :].rearrange("c p n -> p c n"),
                    )
                # the block DMA zeroes partitions 119..126; restore dec rows
                # (W_c at partition 127 is staged by the host)
                if b == 0:
                    for c in range(NCH):
                        nc.sync.dma_start(
                            out=buf[119:127, c, :], in_=dec_wide[:, c, :]
                        )
                else:
                    nc.sync.dma_start(out=buf[119:127, :, :], in_=dec_wide)

                score_cols = smal.tile([CH, NCH], F32, tag="scol")
                score_cols_t[b] = score_cols
                for c in range(NCH):
                    m = chunk_m(c)
                    att = psA.tile([CH, N], F32, tag="att")
                    for h in range(2):
                        nc.tensor.matmul(
                            att[0:m, h * 512:(h + 1) * 512],
                            lhsT=lhs_all[:, b * W + c * CH: b * W + c * CH + m],
                            rhs=buf[:, c, h * 512:(h + 1) * 512],
                            start=True, stop=True, skip_group_check=True,
                        )
                    th = thp.tile([CH, N], BF16, tag="th")
                    nc.scalar.activation(th[0:m, :], att[0:m, :], AF.Tanh)
                    scr = ttro.tile([CH, N], BF16, tag="ttro")
                    nc.vector.scalar_tensor_tensor(
                        out=scr[0:m, :], in0=th[0:m, :], scalar=1.0,
                        in1=v_bcast[0:m, :],
                        op0=ALU.mult, op1=ALU.mult,
                        accum_out=score_cols[0:m, c:c + 1],
                    )

                # score columns -> row b of score_all (t = c*119 + p)
                ps9 = psT.tile([NCH, CH], F32, tag="tscratch")
                nc.tensor.matmul(
                    ps9, lhsT=score_cols, rhs=ident, is_transpose=True,
                    start=True, stop=True,
                )
                score9 = smal.tile([NCH, CH], F32, tag="s9")
                nc.scalar.activation(score9, ps9, AF.Copy)
                srow, _, r = sm_row(b)
                nc.gpsimd.dma_start(
                    out=srow[r:r + 1, 0:(NCH - 1) * CH].rearrange(
                        "p (c w) -> p c w", c=NCH - 1),
                    in_=score9[0:NCH - 1, :],
                )
                nc.gpsimd.dma_start(
                    out=srow[r:r + 1, (NCH - 1) * CH:T],
                    in_=score9[NCH - 1:NCH, 0:LAST],
                )

            # masked softmax + renorm + coverage for GRP batches at once
            def softmax(score_t, attn_t, mask_t, cov_t, covn_t, nr):
                # scores are O(1) (|s| < ~3): plain exp is safe, skip max-sub
                sl = slice(0, nr)
                nc.scalar.activation(attn_t[sl, :], score_t[sl, :], AF.Exp)
                ssum = smal.tile([GRP, 1], F32, tag="ssum")
                nc.vector.scalar_tensor_tensor(
                    out=attn_t[sl, :], in0=attn_t[sl, :], scalar=1.0,
                    in1=mask_t[sl, :],
                    op0=ALU.mult, op1=ALU.mult, accum_out=ssum[sl, :],
                )
                rs = smal.tile([GRP, 1], F32, tag="rs")
                nc.vector.reciprocal(rs[sl, :], ssum[sl, :])
                nc.vector.tensor_scalar_mul(attn_t[sl, :], attn_t[sl, :],
                                            rs[sl, :])
                nc.vector.tensor_add(covn_t[sl, :], cov_t[sl, :],
                                     attn_t[sl, :])

            # ---------------- pass 2: context vectors ----------------
            eo_bufs = {}

            def load_eo(b):
                buf = eo_bufs_pool[b % 3]
                nc.sync.dma_start(
                    out=buf[:, :, :],
                    in_=eo_d[b, :, :, :].rearrange("c p n -> p c n"),
                )
                eo_bufs[b] = buf

            ctps = [psS.tile([NB, 512], F32, tag="srow", name=f"ctp{h}")
                    for h in range(2)]

            def phase_c(b):
                # attn row -> [8, 128] -> transpose -> one-hot column b
                # (c_t is free of the 119-chunking: EO uses natural blocks)
                attn8 = a9p.tile([8, 128], F32, tag="attn9")
                _, arow, r = sm_row(b)
                nc.sync.dma_start(
                    out=attn8,
                    in_=arow[r:r + 1, :].rearrange("p (j t) -> p j t", j=8),
                )
                acp = psT.tile([128, 8], F32, tag="tscratch")
                nc.tensor.matmul(
                    acp, lhsT=attn8, rhs=ident[0:8, 0:8],
                    is_transpose=True, start=True, stop=True,
                )
                acw = acwp.tile([128, 8, NB], BF16, tag="acw")
                nc.gpsimd.memset(acw, 0.0)
                nc.scalar.activation(acw[:, :, b], acp, AF.Copy)

                buf = eo_bufs.pop(b)
                for c in range(8):
                    for h in range(2):
                        nc.tensor.matmul(
                            ctps[h],
                            lhsT=acw[:, c, :],
                            rhs=buf[:, c, h * 512:(h + 1) * 512],
                            start=(b == 0 and c == 0),
                            stop=(b == nb - 1 and c == 7),
                            skip_group_check=True,
                        )

            # ---------------- schedule ----------------
            # softmax split 0-3 / 4-5 / 6-7: C(4),C(5) depend only on the
            # early 4-5 softmax, so the post-A(7) barrier is just batches
            # 6-7's row ops, fully hidden under C(4)/C(5)
            for b in range(4):
                phase_a(b)
                if b >= 2:
                    load_eo(b - 2)
            softmax(score_A, attn_A, mask_A, cov_A, covn_A, 4)
            # rows 0-3 flush now so set A can be reused for batches 6-7
            nc.sync.dma_start(out=at_o[0:4, :], in_=attn_A)
            nc.sync.dma_start(out=cn_o[0:4, :], in_=covn_A)
            phase_a(4)
            phase_c(0)
            load_eo(2)
            phase_a(5)
            phase_c(1)
            load_eo(3)
            softmax(score_B, attn_B, mask_B, cov_B, covn_B, 2)
            phase_a(6)
            phase_c(2)
            load_eo(4)
            phase_a(7)
            phase_c(3)
            load_eo(5)
            load_eo(6)
            softmax(score_A, attn_A, mask_C, cov_C, covn_A, 2)
            phase_c(4)
            load_eo(7)
            for b in range(5, nb):
                phase_c(b)
            nc.sync.dma_start(out=at_o[4:6, :], in_=attn_B)
            nc.sync.dma_start(out=cn_o[4:6, :], in_=covn_B)
            nc.sync.dma_start(out=at_o[6:8, :], in_=attn_A[0:2, :])
            nc.sync.dma_start(out=cn_o[6:8, :], in_=covn_A[0:2, :])

            for h in range(2):
                nc.scalar.activation(
                    ct_sb[:, h * 512:(h + 1) * 512], ctps[h], AF.Copy
                )
            nc.sync.dma_start(out=ct_o[:, :], in_=ct_sb)

    nc.finalize()
    return nc


_CACHE: dict = {}


def _get_nc() -> bass.Bass:
    if "nc" not in _CACHE:
        _CACHE["nc"] = build_bass(NB)
    return _CACHE["nc"]


def _stage_tiles(x, row127=None, dtype=ml_dtypes.bfloat16):
    """[T, N] f32 -> [9, 128, N] blocks: block c rows 0..118 hold
    t = c*119 + p (zero-padded past T); rows 119..126 zero; row 127 =
    `row127` if given (W_c for the EF operand).  DMA-friendly: every
    transfer writes all 128 partitions with p-minor lines."""
    blk = np.zeros((NCH, 128, N), np.float32)
    pad = np.zeros((CH * NCH, N), np.float32)
    pad[:T] = x
    blk[:, 0:CH, :] = pad.reshape(NCH, CH, N)
    if row127 is not None:
        blk[:, 127, :] = row127[None, :]
    return blk.astype(dtype)


def _build_lhs(cov_core):
    """identity band + one-hot rows + cov row, split into 4KB-line part A
    ([4, 128, 2048], cols 0..8191 pair-packed) and tail part B."""
    lhs = np.zeros((128, NB * W), np.float32)
    eye = np.eye(CH, dtype=np.float32)
    lhs[:CH] = np.tile(eye, (1, NB * NCH))
    for j in range(NB):
        lhs[CH + j, j * W:(j + 1) * W] = 1.0
        lhs[127, j * W:j * W + T] = cov_core[j]
    lhs = lhs.astype(ml_dtypes.float8_e4m3)
    a = np.ascontiguousarray(lhs[:, 0:8192].reshape(128, 4, 2048).transpose(1, 0, 2))
    b = np.ascontiguousarray(lhs[:, 8192:])
    return a, b


def make_in_maps(inputs: dict) -> list:
    f = lambda x: np.ascontiguousarray(np.asarray(x), dtype=np.float32)
    s = f(inputs["s_t_hat"])
    eo = f(inputs["encoder_outputs"])
    ef = f(inputs["encoder_feature"]).reshape(B, T, N)
    mk = f(inputs["enc_padding_mask"])
    cv = f(inputs["coverage"])
    wdt = np.ascontiguousarray(f(inputs["W_d"]).T).astype(ml_dtypes.bfloat16)
    # [1024, 1024] -> k-tile pair layout [4, 128, 2048] (4KB DMA lines):
    # wdt_all[p, kj, :] = W_d_T[kj*128 + p, :]
    wdt_pairs = np.ascontiguousarray(
        wdt.reshape(KT, 128, N).transpose(1, 0, 2).reshape(128, 4, 2048)
        .transpose(1, 0, 2)
    )
    bd = f(inputs["b_d"])
    wc = f(inputs["W_c"])
    vv = f(inputs["v"])
    in_maps = []
    for i in range(N_CORES):
        sl = slice(i * NB, (i + 1) * NB)
        ef_blk = np.stack([_stage_tiles(ef[i * NB + j], row127=wc,
                                        dtype=ml_dtypes.float8_e4m3)
                           for j in range(NB)])
        eo_blk = np.ascontiguousarray(
            eo[sl].reshape(NB, 8, 128, N)).astype(ml_dtypes.bfloat16)
        lhs_a, lhs_b = _build_lhs(cv[sl])
        in_maps.append({
            "ef_blk": ef_blk,
            "eo_blk": eo_blk,
            "lhsT_a": lhs_a,
            "lhsT_b": lhs_b,
            "s_t_hat_T": np.ascontiguousarray(s[sl].T).astype(ml_dtypes.bfloat16),
            "enc_padding_mask": mk[sl],
            "coverage": cv[sl],
            "W_d_T": wdt_pairs,
            "b_d": bd.astype(ml_dtypes.bfloat16),
            "W_c": wc.astype(ml_dtypes.bfloat16),
            "v": vv.astype(ml_dtypes.bfloat16),
        })
    return in_maps


def gather_outputs(results: list):
    c_t = np.concatenate([results[i]["c_t"] for i in range(N_CORES)], axis=0)
    attn = np.concatenate([results[i]["attn"] for i in range(N_CORES)], axis=0)
    covn = np.concatenate(
        [results[i]["coverage_next"] for i in range(N_CORES)], axis=0
    )
    return c_t, attn, covn


def kernel(**inputs):
    nc = _get_nc()
    in_maps = make_in_maps(inputs)
    res = run_bass_kernel_spmd(nc, in_maps, core_ids=list(range(N_CORES)))
    return gather_outputs(res.results)
